# revision 1
# baseline (speedup 1.0000x reference)
"""ConcatCritic pair-scorer on 8 TRN2 cores.

reference:  out[a, c] = W2 . relu(concat(x[a], y[c]) @ W1 + b1) + b2
factorized: out[a, c] = W2 . relu(Xp[a, :] + Yp[c, :] + b1) + b2
            with Xp = x @ W1[:D],  Yp = y @ W1[D:]

Sharding: data-parallel over the x/batch rows (64 rows per core). Each core
holds full y, W1, b1, W2 and computes its [64, 512] stripe of the output.
Inputs are fed pre-transposed from the host (xT, yT, b1T, w2T) so the device
does no layout shuffling.

Per-core dataflow (h = hidden index on partitions, 4 h-tiles of 128):
  setup (fp32 on PE):
    Y_kt [128h, 512c] = (y @ W1y)^T tile   via matmul(lhsT=W1y[:,hs], rhs=yT)
    q_kt [128h,  64a] = (x @ W1x)^T + b1   via matmul(lhsT=W1x[:,hs], rhs=xT)
    both rounded to bf16 SBUF tiles.
  main loop (64 output rows in 16 groups of 4, 2 blocks of 8 groups):
    u[s] = relu(Y_kt + q_kt[:, a])  bf16, on DVE (tensor_scalar add+max,
                                    4x mode) or ACT (activation Relu + bias)
    psum_g[32j, 512c] += w2T[:, kt].T @ u   M=1 bf16 matmul; the 4 rows of a
                                    group go to partitions {0,32,64,96} of one
                                    PSUM bank via tile_position=(0, 32j), so
                                    4 matmuls stream concurrently on separate
                                    PE column-groups.
    per group: one ACT/DVE copy PSUM->SBUF (f32), 4 row-DMAs to out.

All tiles are fixed allocations (no pool slot rotation) so cross-engine
slot-release waits never stack up: TPB instructions carry at most ONE sync
wait; Bacc's generate_event_semaphores legalizes any extras into
EventSemaphore instructions, which serialize the engine queues - the layout
below keeps those rare.
"""

from contextlib import ExitStack

import ml_dtypes
import numpy as np

import concourse.bacc as bacc
import concourse.bass as bass
import concourse.mybir as mybir
import concourse.tile as tile
from concourse.bass_utils import run_bass_kernel_spmd

B = 512
D = 128
H = 512
NCORES = 8
BS = B // NCORES  # 64 x-rows per core
KT = H // 128  # 4 h-tiles
GROUPS = BS // 4  # 16 groups of 4 output rows
GPB = 8  # groups per block (8 PSUM banks)
NBLK = GROUPS // GPB
FP = mybir.dt.float32
BF = mybir.dt.bfloat16

_NC = None
LAST_RESULTS = None


def _act_relu(kt: int, g: int, j: int) -> bool:
    """Which relu tiles ScalarE produces (rest go to VectorE).

    (kt==0, j==0) tiles MUST be ScalarE: the first matmul touching a psum
    bank also carries the bank's WAR release (the ScalarE score copy of the
    previous block), and PE matmuls support only one sync wait - producer
    and release must be the same semaphore.  The extra ACT tiles balance
    engine time (ACT ~400ns vs DVE ~194ns per tile).
    """
    if j == 0:
        return True  # 8 per (kt) phase: covers the kt==0 constraint
    return j == 1 and g < 3  # ~3 more per phase for balance


def _copy_engine(gg: int) -> str:
    # split the 16 PSUM->SBUF score copies between ACT and DVE
    return "s" if gg % 2 == 0 else "v"


def _build_nc():
    nc = bacc.Bacc(None, target_bir_lowering=False, num_devices=NCORES)
    xT = nc.dram_tensor("xT", [D, BS], FP, kind="ExternalInput")
    yT = nc.dram_tensor("yT", [D, B], FP, kind="ExternalInput")
    W1 = nc.dram_tensor("W1", [2 * D, H], FP, kind="ExternalInput")
    b1T = nc.dram_tensor("b1T", [128, KT], FP, kind="ExternalInput")
    w2T = nc.dram_tensor("w2T", [128, KT], BF, kind="ExternalInput")
    out = nc.dram_tensor("out", [BS, B], FP, kind="ExternalOutput")

    with tile.TileContext(nc) as tc, ExitStack() as ctx:
        const = ctx.enter_context(tc.tile_pool(name="const", bufs=1))

        W1x_sb = const.tile([128, H], FP, tag="W1x")
        nc.sync.dma_start(W1x_sb[:], W1[0:D, :])
        W1y_sb = const.tile([128, H], FP, tag="W1y")
        nc.sync.dma_start(W1y_sb[:], W1[D : 2 * D, :])
        xT_sb = const.tile([128, BS], FP, tag="xT")
        nc.sync.dma_start(xT_sb[:], xT[:, :])
        yT_sb = const.tile([128, B], FP, tag="yT")
        nc.sync.dma_start(yT_sb[:], yT[:, :])
        b1T_sb = const.tile([128, KT], FP, tag="b1T")
        nc.sync.dma_start(b1T_sb[:], b1T[:, :])
        w2T_sb = const.tile([128, KT], BF, tag="w2T")
        nc.sync.dma_start(w2T_sb[:], w2T[:, :])

        # warm DVE's view of b1T's DMA semaphore (keeps the q_kt tensor_scalar
        # at one wait)
        scratch = const.tile([128, KT], FP, tag="scratch")
        nc.vector.tensor_copy(scratch[:], b1T_sb[:])

        # fixed PSUM tiles: 8 score banks, reused across the 2 blocks
        score_ps = ctx.enter_context(tc.tile_pool(name="score_ps", bufs=1, space="PSUM"))
        ps = [score_ps.tile([128, B], FP, tag=f"ps{g}", name=f"ps{g}") for g in range(GPB)]
        # setup matmuls ping-pong between two dedicated psum tiles; they are
        # only live during setup, then stay unused (SBUF->PSUM budget: 8+0,
        # setup reuses score banks ps[0], ps[1] AFTER warmup? no - separate:
        # warm + setup writes land in ps banks before any score matmul and
        # are WAW-overwritten by the first start=True score matmul (same
        # engine, no extra waits).

        # PE matmuls support only ONE sync wait.  Warm the PE's view of every
        # input-DMA semaphore with tiny self-referencing matmuls (one new
        # semaphore each) so no real matmul ever needs two waits.
        warm_mms = []
        for src in (w2T_sb, W1y_sb, W1x_sb, yT_sb, xT_sb):
            mm = nc.tensor.matmul(
                ps[0][:1, :KT], src[:, 0:1], src[:, 0:KT], start=True, stop=True
            )
            warm_mms.append(mm)

        # Y_kt = (y @ W1y)^T tiles, q_kt = (x @ W1x)^T + b1 (bf16 for the
        # 4x-mode DVE relu; q stays f32 - it is only read as a per-partition
        # scalar)
        Y = []
        Q = []
        first_mm = None
        for kt in range(KT):
            hs = slice(kt * 128, (kt + 1) * 128)
            ps_Y = ps[2 * (kt % 2)]
            mm_Y = nc.tensor.matmul(ps_Y[:], W1y_sb[:, hs], yT_sb[:], start=True, stop=True)
            if first_mm is None:
                first_mm = mm_Y
                for wmm in warm_mms:
                    tile.add_dep_helper(
                        first_mm.ins, wmm.ins, sync=True, reason="PE 1-wait warmup"
                    )
            Ykt = const.tile([128, B], BF, tag=f"Y{kt}")
            nc.vector.tensor_copy(Ykt[:], ps_Y[:])
            Y.append(Ykt)

            ps_q = ps[2 * (kt % 2) + 1]
            nc.tensor.matmul(ps_q[:, :BS], W1x_sb[:, hs], xT_sb[:], start=True, stop=True)
            qkt = const.tile([128, BS], FP, tag=f"q{kt}")
            nc.vector.tensor_scalar(
                qkt[:], ps_q[:, :BS], b1T_sb[:, kt : kt + 1], None, mybir.AluOpType.add
            )
            Q.append(qkt)

        # fixed u tiles (one per (g, j) position in a phase) and output
        # staging tiles (one per group, never reused)
        U = [const.tile([128, B], BF, tag=f"u{s}", name=f"u{s}") for s in range(4 * GPB)]
        SB = [const.tile([128, B], FP, tag=f"o{gg}", name=f"o{gg}") for gg in range(GROUPS)]

        for blk in range(NBLK):
            for kt in range(KT):
                for g in range(GPB):
                    gg = blk * GPB + g
                    for j in range(4):
                        a = gg * 4 + j
                        s = g * 4 + j
                        u = U[s]
                        bias_col = Q[kt][:, a : a + 1]
                        if _act_relu(kt, g, j):
                            nc.scalar.activation(
                                u[:],
                                Y[kt][:],
                                mybir.ActivationFunctionType.Relu,
                                bias=bias_col,
                            )
                        else:
                            nc.vector.tensor_scalar(
                                u[:],
                                Y[kt][:],
                                bias_col,
                                0.0,
                                mybir.AluOpType.add,
                                mybir.AluOpType.max,
                            )
                        nc.tensor.matmul(
                            ps[g][32 * j : 32 * j + 1, :],
                            w2T_sb[:, kt : kt + 1],
                            u[:],
                            start=(kt == 0),
                            stop=(kt == KT - 1),
                            tile_position=(0, 32 * j),
                            skip_group_check=True,
                        )
            for g in range(GPB):
                gg = blk * GPB + g
                sb = SB[gg]
                if _copy_engine(gg) == "s":
                    nc.scalar.copy(sb[:], ps[g][:])
                else:
                    nc.vector.tensor_copy(sb[:], ps[g][:])
                for j in range(4):
                    nc.sync.dma_start(
                        out[gg * 4 + j : gg * 4 + j + 1, :],
                        sb[32 * j : 32 * j + 1, :],
                    )

    nc.finalize()
    return nc


def kernel(**inputs) -> np.ndarray:
    global _NC, LAST_RESULTS
    if _NC is None:
        _NC = _build_nc()

    x = np.asarray(inputs["x"], dtype=np.float32)
    y = np.asarray(inputs["y"], dtype=np.float32)
    W1 = np.ascontiguousarray(inputs["W1"], dtype=np.float32)
    b1 = np.asarray(inputs["b1"], dtype=np.float32)
    W2 = np.asarray(inputs["W2"], dtype=np.float32)
    b2 = np.asarray(inputs["b2"], dtype=np.float32)

    yT = np.ascontiguousarray(y.T)
    b1T = np.ascontiguousarray(b1.reshape(KT, 128).T)
    w2T = np.ascontiguousarray(W2[:, 0].reshape(KT, 128).T.astype(ml_dtypes.bfloat16))

    in_maps = [
        {
            "xT": np.ascontiguousarray(x[m * BS : (m + 1) * BS].T),
            "yT": yT,
            "W1": W1,
            "b1T": b1T,
            "w2T": w2T,
        }
        for m in range(NCORES)
    ]
    LAST_RESULTS = run_bass_kernel_spmd(_NC, in_maps, list(range(NCORES)))
    S = np.concatenate([LAST_RESULTS.results[m]["out"] for m in range(NCORES)], axis=0)
    return (S + b2[0]).astype(np.float32)



# revision 8
# speedup vs baseline: 1.1495x; 1.1495x over previous
"""ConcatCritic pair-scorer on 8 TRN2 cores — fp8-DoubleRow / bf16 hybrid.

reference:  out[a, c] = W2 . relu(concat(x[a], y[c]) @ W1 + b1) + b2
factorized: out[a, c] = W2 . relu(Xp[a, :] + Yp[c, :] + b1) + b2
            with Xp = x @ W1[:D],  Yp = y @ W1[D:]

Sharding: data-parallel over the x/batch rows (64 rows per core). Each core
holds full y, W1, b1, W2 and computes its [64, 512] stripe of the output.

The hidden dim h is PERMUTED on the host so |W2[h]| is ascending; the low
half (h-tiles 0,1 — ~14% of the W2 energy) is evaluated in fp8-e4m3 with a
single DoubleRow matmul per output row (0.5 PE cycles/row), the high half
(h-tiles 2,3) in bf16 (1 cycle/row).  fp8 weights are exponent-only
(sign*2^e, exact in e4m3); the mantissa residue m = |w2|/2^e is folded into
the Y/q setup tiles, so only the relu activations carry fp8 rounding.

Per-core dataflow (h on partitions, 4 h-tiles of 128):
  setup (PE in bf16):
    Y_kt  [128h, 512c] = (y @ W1y)^T tile;  q_kt [128h, 64a] = (x @ W1x)^T
    kt 0,1: Ym = Y*m (bf16), qm = (q + b1)*m (f32)       [fp8 path, scaled]
    kt 0-3: Ybf = Y (bf16),  qb = q + b1 (f32)           [bf16 path]
  main loop, 16 groups of 4 rows; group types (host-tuned mix):
    'B': u2[a] [128,2,512] fp8 = relu(Ym_kt + qm_kt[a]) for kt 0,1
         + 2 bf16 u tiles for kt 2,3; PE: 1 DoubleRow mm + 2 bf16 mms
    'C': 4 bf16 u tiles (kt 0-3), 4 bf16 mms         (all-bf16 row group)
    scores accumulate in one PSUM bank per group at partitions {0,32,64,96}
    via tile_position=(0, 32j); per group one PSUM->SBUF copy (ACT or DVE)
    then 4 row-DMAs to DRAM.

Engine balance targets ~44us on each of PE / DVE / ACT:
  PE : 36 DoubleRow mms (107ns) + 184 bf16 mms (213ns)
  ACT: 32 fp8 pairs (2x612ns) + 8 score copies
  DVE: 4 fp8 pairs, all 184 bf16 relu tiles (4x mode, 194ns), 8 copies

The 1-sync-wait PE rule is kept: each matmul waits only on its u-tile
producer; a PSUM bank's block-1 reuse is safe because the bank's block-0
copy runs on the same engine as (and before) the reusing group's first
u-tile producer.
"""

from contextlib import ExitStack

import ml_dtypes
import numpy as np

import concourse.bacc as bacc
import concourse.bass as bass
import concourse.mybir as mybir
import concourse.tile as tile
from concourse.bass_utils import run_bass_kernel_spmd

B = 512
D = 128
H = 512
NCORES = 8
BS = B // NCORES  # 64 x-rows per core
KT = H // 128  # 4 h-tiles
GROUPS = BS // 4  # 16 groups of 4 output rows
GPB = 8  # groups per block (8 PSUM banks)
FP = mybir.dt.float32
BF = mybir.dt.bfloat16
F8 = mybir.dt.float8e4

# group types: 'B' = fp8 pair (kt 0,1) + bf16 (kt 2,3); 'C' = all bf16
GTYPES = "BCBCBBCBCBBCBCBC"
assert len(GTYPES) == GROUPS
# number of fp8 pairs handled by ACT per B group (rest on DVE); j=0 must be
# ACT so the block-1 bank WAR rides the ACT copy's semaphore
ACT_PAIRS = {}
_b_seq = 0
for _g, _t in enumerate(GTYPES):
    if _t == "B":
        ACT_PAIRS[_g] = 4 if _b_seq % 2 == 0 else 3
        _b_seq += 1
# copy engine per group. banks 0-7 (block 0): must match the first u-tile
# producer of the group that reuses the bank: 'B' groups start with an ACT
# fp8 pair, 'C' groups with a DVE bf16 tile. block 1: free, balance engines.
COPY_ENG = {}
for _g in range(GPB):
    COPY_ENG[_g] = "s" if GTYPES[_g + GPB] == "B" else "v"
for _g in range(GPB, GROUPS):
    COPY_ENG[_g] = "s" if (_g % 2 == 0) else "v"

_NC = None
LAST_RESULTS = None


def _build_nc():
    nc = bacc.Bacc(None, target_bir_lowering=False, num_devices=NCORES)
    xT = nc.dram_tensor("xT", [D, BS], BF, kind="ExternalInput")
    yT = nc.dram_tensor("yT", [D, B], BF, kind="ExternalInput")
    W1x = nc.dram_tensor("W1x", [D, H], BF, kind="ExternalInput")
    W1y = nc.dram_tensor("W1y", [D, H], BF, kind="ExternalInput")
    b1T = nc.dram_tensor("b1T", [128, KT], FP, kind="ExternalInput")
    w2bf = nc.dram_tensor("w2bf", [128, KT], BF, kind="ExternalInput")
    # DoubleRow weights: 4 slices of [128, 2, 128]; slice j has the w2 pair
    # in column 32j (score lands on psum partition 32j), zeros elsewhere.
    # dual-fp8 LdWeights requires the full 128-wide PE tile, so M=128 with
    # zero padding (PE time only depends on the moving dim, N=512).
    w2dr = nc.dram_tensor("w2dr", [128, 2 * 512], F8, kind="ExternalInput")
    mcol = nc.dram_tensor("mcol", [128, 2], FP, kind="ExternalInput")
    out = nc.dram_tensor("out", [BS, B], FP, kind="ExternalOutput")

    RELU = mybir.ActivationFunctionType.Relu
    ADD = mybir.AluOpType.add
    MAX = mybir.AluOpType.max
    MULT = mybir.AluOpType.mult

    with tile.TileContext(nc) as tc, ExitStack() as ctx:
        const = ctx.enter_context(tc.tile_pool(name="const", bufs=1))

        W1x_sb = const.tile([128, H], BF, tag="W1x")
        nc.sync.dma_start(W1x_sb[:], W1x[:, :])
        W1y_sb = const.tile([128, H], BF, tag="W1y")
        nc.sync.dma_start(W1y_sb[:], W1y[:, :])
        xT_sb = const.tile([128, BS], BF, tag="xT")
        nc.sync.dma_start(xT_sb[:], xT[:, :])
        yT_sb = const.tile([128, B], BF, tag="yT")
        nc.sync.dma_start(yT_sb[:], yT[:, :])
        b1T_sb = const.tile([128, KT], FP, tag="b1T")
        nc.sync.dma_start(b1T_sb[:], b1T[:, :])
        w2bf_sb = const.tile([128, KT], BF, tag="w2bf")
        nc.sync.dma_start(w2bf_sb[:], w2bf[:, :])
        w2dr_sb = const.tile([128, 2, 512], F8, tag="w2dr")
        nc.sync.dma_start(w2dr_sb[:], w2dr[:, :])
        mcol_sb = const.tile([128, 2], FP, tag="mcol")
        nc.sync.dma_start(mcol_sb[:], mcol[:, :])

        # warm DVE's view of the input-DMA semaphore (mcol is the last DMA)
        scratch = const.tile([128, 2], FP, tag="scratch")
        nc.vector.tensor_copy(scratch[:], mcol_sb[:])

        # fixed PSUM tiles: 8 score banks (setup reuses them before the loop)
        score_ps = ctx.enter_context(tc.tile_pool(name="score_ps", bufs=1, space="PSUM"))
        ps = [score_ps.tile([128, B], FP, tag=f"ps{g}", name=f"ps{g}") for g in range(GPB)]

        # PE matmuls support only ONE sync wait.  Warm the PE's view of every
        # input-DMA semaphore with tiny self-referencing matmuls.
        warm_mms = []
        for src in (w2bf_sb, W1y_sb, W1x_sb, yT_sb, xT_sb):
            mm = nc.tensor.matmul(
                ps[0][:1, :2], src[:, 0:1], src[:, 0:2], start=True, stop=True
            )
            warm_mms.append(mm)
        mm = nc.tensor.matmul(
            ps[0][:1, :2], w2dr_sb[:, 0, 0:1], w2dr_sb[:, 0, 0:2],
            start=True, stop=True,
        )
        warm_mms.append(mm)

        # setup: Y_kt into banks 0-3, q_kt into banks 4-7 (all bf16 matmuls)
        Ym = []  # kt 0,1: bf16, scaled by m
        Ybf = []  # kt 0-3: bf16, unscaled
        qm = []  # kt 0,1: f32, (q+b1)*m
        qb = []  # kt 0-3: f32, q+b1
        first_mm = None
        for kt in range(KT):
            hs = slice(kt * 128, (kt + 1) * 128)
            mm_Y = nc.tensor.matmul(ps[kt][:], W1y_sb[:, hs], yT_sb[:], start=True, stop=True)
            if first_mm is None:
                first_mm = mm_Y
                for wmm in warm_mms:
                    tile.add_dep_helper(
                        first_mm.ins, wmm.ins, sync=True, reason="PE 1-wait warmup"
                    )
            nc.tensor.matmul(
                ps[4 + kt][:, :BS], W1x_sb[:, hs], xT_sb[:], start=True, stop=True
            )

        for kt in range(KT):
            ybf = const.tile([128, B], BF, tag=f"Ybf{kt}")
            nc.scalar.copy(ybf[:], ps[kt][:])
            Ybf.append(ybf)
            qbk = const.tile([128, BS], FP, tag=f"qb{kt}")
            nc.vector.tensor_scalar(
                qbk[:], ps[4 + kt][:, :BS], b1T_sb[:, kt : kt + 1], None, ADD
            )
            qb.append(qbk)
        for kt in range(2):
            ym = const.tile([128, B], BF, tag=f"Ym{kt}")
            nc.vector.tensor_scalar(
                ym[:], ps[kt][:], mcol_sb[:, kt : kt + 1], None, MULT
            )
            Ym.append(ym)
            qmk = const.tile([128, BS], FP, tag=f"qm{kt}")
            nc.vector.tensor_scalar(
                qmk[:],
                ps[4 + kt][:, :BS],
                b1T_sb[:, kt : kt + 1],
                mcol_sb[:, kt : kt + 1],
                ADD,
                MULT,
            )
            qm.append(qmk)

        # u slots: 2 groups in flight
        U2 = [const.tile([128, 2, B], F8, tag=f"u2_{s}", name=f"u2_{s}") for s in range(8)]
        UB = [const.tile([128, B], BF, tag=f"ub_{s}", name=f"ub_{s}") for s in range(32)]
        SB = [const.tile([128, B], FP, tag=f"o{g}", name=f"o{g}") for g in range(GROUPS)]

        def emit_ew(g):
            gt = GTYPES[g]
            if gt == "B":
                nact = ACT_PAIRS[g]
                # ACT pairs first (j < nact), then DVE pairs, then DVE bf16
                for j in range(4):
                    a = g * 4 + j
                    u2 = U2[(g % 2) * 4 + j]
                    if j < nact:
                        for k2 in range(2):
                            nc.scalar.activation(
                                u2[:, k2, :], Ym[k2][:], RELU,
                                bias=qm[k2][:, a : a + 1],
                            )
                for j in range(nact, 4):
                    a = g * 4 + j
                    u2 = U2[(g % 2) * 4 + j]
                    for k2 in range(2):
                        nc.vector.tensor_scalar(
                            u2[:, k2, :], Ym[k2][:], qm[k2][:, a : a + 1],
                            0.0, ADD, MAX,
                        )
                for kt in (2, 3):
                    for j in range(4):
                        a = g * 4 + j
                        ub = UB[(g % 2) * 16 + (kt - 2) * 4 + j]
                        nc.vector.tensor_scalar(
                            ub[:], Ybf[kt][:], qb[kt][:, a : a + 1], 0.0, ADD, MAX
                        )
            else:
                for kt in range(KT):
                    for j in range(4):
                        a = g * 4 + j
                        ub = UB[(g % 2) * 16 + kt * 4 + j]
                        nc.vector.tensor_scalar(
                            ub[:], Ybf[kt][:], qb[kt][:, a : a + 1], 0.0, ADD, MAX
                        )

        def emit_mm(g):
            gt = GTYPES[g]
            bank = ps[g % GPB]
            if gt == "B":
                for j in range(4):
                    u2 = U2[(g % 2) * 4 + j]
                    nc.tensor.matmul(
                        bank[:, :],
                        w2dr_sb[:, :, 128 * j : 128 * (j + 1)],
                        u2[:, :, :],
                        start=(j == 0),
                        stop=False,
                        perf_mode=mybir.MatmulPerfMode.DoubleRow,
                        skip_group_check=True,
                    )
                for kt in (2, 3):
                    for j in range(4):
                        ub = UB[(g % 2) * 16 + (kt - 2) * 4 + j]
                        nc.tensor.matmul(
                            bank[32 * j : 32 * j + 1, :],
                            w2bf_sb[:, kt : kt + 1],
                            ub[:],
                            start=False,
                            stop=(kt == 3),
                            tile_position=(0, 32 * j),
                            skip_group_check=True,
                        )
            else:
                for kt in range(KT):
                    for j in range(4):
                        ub = UB[(g % 2) * 16 + kt * 4 + j]
                        nc.tensor.matmul(
                            bank[32 * j : 32 * j + 1, :],
                            w2bf_sb[:, kt : kt + 1],
                            ub[:],
                            start=(kt == 0),
                            stop=(kt == 3),
                            tile_position=(0, 32 * j),
                            skip_group_check=True,
                        )

        def emit_out(g):
            sb = SB[g]
            if COPY_ENG[g] == "s":
                nc.scalar.copy(sb[:], ps[g % GPB][:])
            else:
                nc.vector.tensor_copy(sb[:], ps[g % GPB][:])
            for j in range(4):
                nc.sync.dma_start(
                    out[g * 4 + j : g * 4 + j + 1, :],
                    sb[32 * j : 32 * j + 1, :],
                )

        for t in range(GROUPS + 2):
            if t < GROUPS:
                emit_ew(t)
            if 1 <= t <= GROUPS:
                emit_mm(t - 1)
            if t >= 2:
                emit_out(t - 2)

    nc.finalize()
    return nc


def kernel(**inputs) -> np.ndarray:
    global _NC, LAST_RESULTS
    if _NC is None:
        _NC = _build_nc()

    x = np.asarray(inputs["x"], dtype=np.float32)
    y = np.asarray(inputs["y"], dtype=np.float32)
    W1 = np.ascontiguousarray(inputs["W1"], dtype=np.float32)
    b1 = np.asarray(inputs["b1"], dtype=np.float32)
    W2 = np.asarray(inputs["W2"], dtype=np.float32)
    b2 = np.asarray(inputs["b2"], dtype=np.float32)

    bf = ml_dtypes.bfloat16
    f8 = ml_dtypes.float8_e4m3

    # permute hidden dim: |w2| ascending; low half -> fp8 tiles 0,1
    w2 = W2[:, 0]
    perm = np.argsort(np.abs(w2), kind="stable")
    W1p = W1[:, perm]
    b1p = b1[perm]
    w2p = w2[perm]

    lo = w2p[:256]
    mag = np.abs(lo)
    e = np.where(mag > 0, np.floor(np.log2(np.maximum(mag, 2.0**-9))), -9.0)
    e = np.clip(e, -9.0, 7.0)
    pw = np.sign(lo) * (2.0**e)  # exact in e4m3
    m = np.where(mag > 0, mag / (2.0**e), 0.0).astype(np.float32)

    pwT = pw.reshape(2, 128).T  # [k, k2]
    w2dr_h = np.zeros((128, 2, 512), np.float32)
    for j in range(4):
        # slice j = cols [128j, 128j+128); w2 pair in slice-column 32j
        w2dr_h[:, :, 128 * j + 32 * j] = pwT
    w2dr_h = np.ascontiguousarray(w2dr_h.reshape(128, 1024).astype(f8))
    mcol_h = np.ascontiguousarray(m.reshape(2, 128).T.astype(np.float32))
    w2bf_h = np.ascontiguousarray(w2p.reshape(KT, 128).T.astype(bf))
    b1T_h = np.ascontiguousarray(b1p.reshape(KT, 128).T.astype(np.float32))
    W1x_h = np.ascontiguousarray(W1p[:D].astype(bf))
    W1y_h = np.ascontiguousarray(W1p[D:].astype(bf))
    yT_h = np.ascontiguousarray(y.T.astype(bf))

    in_maps = [
        {
            "xT": np.ascontiguousarray(x[c * BS : (c + 1) * BS].T.astype(bf)),
            "yT": yT_h,
            "W1x": W1x_h,
            "W1y": W1y_h,
            "b1T": b1T_h,
            "w2bf": w2bf_h,
            "w2dr": w2dr_h,
            "mcol": mcol_h,
        }
        for c in range(NCORES)
    ]
    LAST_RESULTS = run_bass_kernel_spmd(_NC, in_maps, list(range(NCORES)))
    S = np.concatenate([LAST_RESULTS.results[c]["out"] for c in range(NCORES)], axis=0)
    return (S + b2[0]).astype(np.float32)
